# revision 28
# baseline (speedup 1.0000x reference)
"""Trainium2 Bass kernel for nn_Attention_64819646431478.

Single-layer causal attention, B=1, T=2048, DIM=1024, 16 heads, head_dim=64,
f32, with RMSNorm (eps=f32 eps) on Q and K heads.

Sharding: tensor-parallel over heads across 8 NeuronCores (2 heads/core).
Each core computes its heads' Q/K/V projections, causal attention, and the
partial output projection against its 128-row slice of w_o; the host sums
the 8 bf16 partial outputs (the "all-reduce" of the hint, at gather time).

v2 layout (fused pipeline; ~2x over the phase-split baseline):
  - One fused loop, software-pipelined: iteration i runs QKV projections of
    chunk c=i, attention of chunk a=i-1, and w_o of chunk w=i-2, so the PE
    never idles at a phase boundary (idle >3.4us re-throttles HAM to 1.2GHz).
  - Single ACT table set (natural_log_exp_and_others): rsqrt for RMSNorm is
    exp(-0.5*ln(mean+eps)); softmax reciprocal is exp(-ln(sum)); plus Square,
    Exp, Copy. No DVE reciprocal passes, no table reloads mid-kernel.
  - Causal band is ragged: diagonal r-tile s covers only tq>=128*s (widths
    512/384/256/128), packed into 1.25 PSUM score tiles; only the four
    128x128 blocks on the true diagonal get a triangle mask (DVE, bf16).
  - Scores are computed transposed per head: ST[tk,tq] = K^ @ Q^, exp needs
    no max-subtraction (|scores|<=8 after RMSNorm).
  - Softmax denominator rides along as V's 65th lhsT column (ones), landing
    in ot row 64 of the PV accumulation.
  - V is transposed by the DMA crossbar (dma_start_transpose), not the PE.
  - PSUM budget exactly 8 banks: st [128,1024]x2 (scores + w_o pairs),
    acc [*,512]x3 (sums/ot0/ot1/b2/bb_q/bb_k rotation), pp [128,512]x1.
  - Output partials are written bf16 (host accumulates in f32).
"""

import os
import sys
import types

import numpy as np

# --- environment bootstrap (harness may run us from a bare directory) ---
for _p in ("/root/.axon_site", "/root/.axon_site/_ro/trn_rl_repo",
           "/root/.axon_site/_ro/pypackages", "/opt/trn_rl_repo"):
    if os.path.isdir(_p) and _p not in sys.path:
        sys.path.append(_p)


def _install_ntff_shim():
    """Provide antenv.axon_hooks (missing in this image) so trace=True works."""
    if "antenv.axon_hooks" in sys.modules:
        return
    mod = types.ModuleType("antenv.axon_hooks")
    mod._hook = None
    mod.set_axon_ntff_profile_hook = lambda h: setattr(mod, "_hook", h)
    mod.get_axon_ntff_profile_hook = lambda: mod._hook
    sys.modules["antenv.axon_hooks"] = mod
    try:
        import antenv
        antenv.axon_hooks = mod
        from trn_agent_boot.trn_boot import _ntff_profile_via_ctypes
        mod.set_axon_ntff_profile_hook(
            _ntff_profile_via_ctypes("/opt/axon/libaxon_pjrt.so"))
    except Exception:
        pass


_install_ntff_shim()

import ml_dtypes  # noqa: E402

import concourse.mybir as mybir  # noqa: E402
import concourse.tile as tile  # noqa: E402
from concourse import bacc  # noqa: E402

F32 = mybir.dt.float32
BF16 = mybir.dt.bfloat16
NP_BF16 = ml_dtypes.bfloat16
AF = mybir.ActivationFunctionType

_TABLES_PATCHED = False


def _pin_act_table_set():
    """Make the ACT table-load chooser resolve Exp/Ln/Square/Copy to the one
    set that holds all four (natural_log_exp_and_others). The default
    per-function primary sets differ (exp_and_others vs natural_log), so a
    kernel alternating Exp and Ln reloads tables every few instructions
    (~2.7us each, 33 loads measured). Stripping these funcs from every other
    set — order and indices preserved — leaves the chooser exactly one valid
    set, so it emits a single load."""
    global _TABLES_PATCHED
    if _TABLES_PATCHED:
        return
    import functools

    import concourse.bacc as bacc_mod
    from concourse.hw_specs import get_activation_tables as _orig

    keep = {AF.Exp, AF.Ln, AF.Square, AF.Copy}
    target = "natural_log_exp_and_others"

    @functools.lru_cache(maxsize=None)
    def patched(arch):
        tabs = _orig(arch)
        if target not in tabs or not keep.issubset(tabs[target]):
            return tabs
        return {name: (funcs if name == target else funcs - keep)
                for name, funcs in tabs.items()}

    bacc_mod.get_activation_tables = patched
    _TABLES_PATCHED = True

T = 2048
C = 1024
D = 64
NCORES = 8
HPC = 2            # heads per core
JPC = HPC * D      # 128 j-columns per core
NTQ = 4            # tq chunks of 512
TQ = 512
EPS = float(np.finfo(np.float32).eps)
SCALE = float(D) ** -0.5


def build_nc():
    _pin_act_table_set()
    nc = bacc.Bacc("TRN2", target_bir_lowering=False, debug=False,
                   num_devices=NCORES)

    xT_d = nc.dram_tensor("xT", [C, T], BF16, kind="ExternalInput")
    wqkv_d = nc.dram_tensor("wqkv", [C, 3 * JPC], BF16, kind="ExternalInput")
    wo_d = nc.dram_tensor("wo", [JPC, C], BF16, kind="ExternalInput")
    selq_d = nc.dram_tensor("selq", [2, 128], BF16, kind="ExternalInput")
    selk_d = nc.dram_tensor("selk", [2, 128], BF16, kind="ExternalInput")
    sel2_d = nc.dram_tensor("sel2", [64, 128], BF16, kind="ExternalInput")
    onescol_d = nc.dram_tensor("onescol", [128, 2], BF16, kind="ExternalInput")
    tri_d = nc.dram_tensor("tri", [128, 128], BF16, kind="ExternalInput")
    vones_d = nc.dram_tensor("vones", [128, 32], BF16, kind="ExternalInput")
    outT_d = nc.dram_tensor("outT", [C, T], BF16, kind="ExternalOutput")

    with tile.TileContext(nc) as tc, nc.allow_low_precision("bf16 kernel"):
        from contextlib import ExitStack
        with ExitStack() as ctx:
            consts = ctx.enter_context(tc.tile_pool(name="consts", bufs=1))
            acts = ctx.enter_context(tc.tile_pool(name="acts", bufs=1))

            # ---- constants to SBUF ----
            wsb = consts.tile([128, 8, 3 * JPC], BF16)
            nc.gpsimd.dma_start(
                out=wsb[:], in_=wqkv_d.rearrange("(c p) j -> p c j", p=128))
            wo_sb = consts.tile([128, C], BF16)
            nc.sync.dma_start(out=wo_sb[:], in_=wo_d[:])
            selq_sb = consts.tile([2, 128], BF16)
            nc.sync.dma_start(out=selq_sb[:], in_=selq_d[:])
            selk_sb = consts.tile([2, 128], BF16)
            nc.sync.dma_start(out=selk_sb[:], in_=selk_d[:])
            sel2_sb = consts.tile([64, 128], BF16)
            nc.sync.dma_start(out=sel2_sb[:], in_=sel2_d[:])
            onescol_sb = consts.tile([128, 2], BF16)
            nc.sync.dma_start(out=onescol_sb[:], in_=onescol_d[:])
            tri_sb = consts.tile([128, 128], BF16)
            nc.sync.dma_start(out=tri_sb[:], in_=tri_d[:])
            eps_sb = consts.tile([2, 1], F32)
            nc.vector.memset(eps_sb[:], EPS)
            zero_sb = consts.tile([1, 1], F32)
            nc.vector.memset(zero_sb[:], 0.0)

            # ---- persistent activations ----
            xT_sb = acts.tile([128, 8, T], BF16)
            QTn = acts.tile([128, T], BF16)
            KTn = acts.tile([128, T], BF16)
            V_sb = acts.tile([128, HPC, 16, 65], BF16)  # [tk%128, h, r, d|1]
            ctxU = acts.tile([128, T], BF16)   # unnormalized ctx^T
            ctxT = acts.tile([128, T], BF16)   # normalized ctx^T
            rec_q = acts.tile([2, T], BF16)    # 1/rms per q head
            rec_k = acts.tile([2, T], BF16)    # 1/rms per k head
            # 1/softmax-denominator: h0 at row 0, h1 at row 32 (engine
            # partition starts must be 32-aligned); other rows zeroed.
            recs = acts.tile([64, T], BF16)
            nc.vector.memset(recs[:], 0.0)

            # ones column of each V tile (lhsT col 64 -> softmax sums row)
            nc.sync.dma_start(
                out=V_sb[:, :, :, 64:65],
                in_=vones_d.rearrange("p (h r u) -> p h r u", h=HPC, u=1))

            # input x, chunk-major so chunk 0 projections start early;
            # split across two DMA queues to halve the ramp
            xT_r = xT_d.rearrange("(c p) t -> p c t", p=128)
            for c in range(NTQ):
                sl = slice(TQ * c, TQ * (c + 1))
                for ci in range(8):
                    eng = nc.gpsimd if ci % 2 == 0 else nc.sync
                    eng.dma_start(out=xT_sb[:, ci, sl], in_=xT_r[:, ci, sl])

            outT_r = outT_d.rearrange("(m p) t -> p m t", p=128)

            with (
                tc.tile_pool(name="vtp", bufs=2) as vtp,
                tc.tile_pool(name="sqp", bufs=2) as sqp,
                tc.tile_pool(name="rawp", bufs=2) as rawp,
                tc.tile_pool(name="lnp", bufs=2) as lnp,
                tc.tile_pool(name="ep", bufs=3) as ep,
                tc.tile_pool(name="stgp", bufs=2) as stgp,
                tc.tile_pool(name="ps_st", bufs=2, space="PSUM") as ps_st,
                tc.tile_pool(name="ps_acc", bufs=2, space="PSUM") as ps_acc,
                tc.tile_pool(name="ps_ot", bufs=2, space="PSUM") as ps_ot,
            ):
                def proj(jbase, csl, name):
                    pp = ps_acc.tile([128, TQ], F32, tag="acc", name=name)
                    for ci in range(8):
                        nc.tensor.matmul(
                            pp[:], wsb[:, ci, jbase:jbase + 128],
                            xT_sb[:, ci, csl], start=(ci == 0), stop=(ci == 7))
                    return pp

                for i in range(NTQ + 2):
                    c = i if i < NTQ else None
                    a = i - 1 if 1 <= i <= NTQ else None
                    w = i - 2 if i >= 2 else None
                    csl = slice(TQ * c, TQ * (c + 1)) if c is not None else None
                    asl = slice(TQ * a, TQ * (a + 1)) if a is not None else None
                    wsl = slice(TQ * w, TQ * (w + 1)) if w is not None else None

                    # PE work is emitted as an interleave of ACT-paced
                    # attention groups and self-sufficient matmul blocks
                    # (projections of chunk c, w_o of chunk w), so the PE
                    # queue never waits on a bursty ACT backlog.
                    blocks = []   # non-attention PE blocks
                    groups = []   # attention groups (score+exp+PV)
                    state = {}

                    if c is not None:
                        def proj_q(c=c, csl=csl):
                            state["pp_q"] = proj(0, csl, f"ppq{c}")
                            raw_q = rawp.tile([128, TQ], BF16, tag="raw",
                                              name="rawq")
                            nc.vector.tensor_copy(raw_q[:], state["pp_q"][:])
                            sq_q = sqp.tile([128, TQ], BF16, tag="sq",
                                            name="sqq")
                            nc.vector.tensor_mul(sq_q[:], raw_q[:], raw_q[:])
                            state["raw_q"], state["sq_q"] = raw_q, sq_q

                        def proj_k(c=c, csl=csl):
                            state["pp_k"] = proj(JPC, csl, f"ppk{c}")
                            raw_k = rawp.tile([128, TQ], BF16, tag="raw",
                                              name="rawk")
                            nc.vector.tensor_copy(raw_k[:], state["pp_k"][:])
                            sq_k = sqp.tile([128, TQ], BF16, tag="sq",
                                            name="sqk")
                            nc.vector.tensor_mul(sq_k[:], raw_k[:], raw_k[:])
                            state["raw_k"], state["sq_k"] = raw_k, sq_k

                        def proj_v(c=c, csl=csl):
                            pp_v = proj(2 * JPC, csl, f"ppv{c}")
                            vt = vtp.tile([128, TQ], BF16, tag="vt",
                                          name=f"vt{c}")
                            nc.vector.tensor_copy(vt[:], pp_v[:])
                            for rl in range(4):
                                # xbar transpose needs a plain contiguous
                                # dest; DVE splits the heads into V_sb
                                vx = vtp.tile([128, 128], BF16, tag="vx",
                                              name=f"vx{c}_{rl}")
                                nc.sync.dma_start_transpose(
                                    out=vx[:],
                                    in_=vt[:, 128 * rl:128 * (rl + 1)])
                                for h in range(HPC):
                                    nc.vector.tensor_copy(
                                        V_sb[:, h, 4 * c + rl, 0:64],
                                        vx[:, 64 * h:64 * (h + 1)])

                        def sums_blk(c=c, csl=csl):
                            # rms: rec = (mean+eps)^-1/2 = exp(-ln/2)
                            sums_q = ps_acc.tile([2, TQ], F32, tag="acc",
                                                 name=f"sumsq{c}")
                            nc.tensor.matmul(sums_q[:], onescol_sb[:],
                                             state["sq_q"][:],
                                             start=True, stop=True)
                            sums_k = ps_acc.tile([2, TQ], F32, tag="acc",
                                                 name=f"sumsk{c}")
                            nc.tensor.matmul(sums_k[:], onescol_sb[:],
                                             state["sq_k"][:],
                                             start=True, stop=True)
                            ln_q = lnp.tile([2, TQ], F32, tag="lns",
                                            name="lnq")
                            nc.scalar.activation(ln_q[:], sums_q[:], AF.Ln,
                                                 bias=eps_sb[:], scale=1.0 / D)
                            nc.scalar.activation(rec_q[:, csl], ln_q[:],
                                                 AF.Exp, scale=-0.5)
                            ln_k = lnp.tile([2, TQ], F32, tag="lns",
                                            name="lnk")
                            nc.scalar.activation(ln_k[:], sums_k[:], AF.Ln,
                                                 bias=eps_sb[:], scale=1.0 / D)
                            nc.scalar.activation(rec_k[:, csl], ln_k[:],
                                                 AF.Exp, scale=-0.5)

                        blocks += [proj_q, proj_k, proj_v, sums_blk]

                    if w is not None:
                        wstate = {}

                        def wo_mm(pair, half, wsl=wsl, w=w, wstate=wstate):
                            mu = 2 * pair + half
                            if half == 0:
                                wstate[pair] = ps_st.tile(
                                    [128, 2 * TQ], F32, tag="st",
                                    name=f"wp{w}_{pair}")
                            wp = wstate[pair]
                            nc.tensor.matmul(
                                wp[:, TQ * half:TQ * (half + 1)],
                                wo_sb[:, 128 * mu:128 * (mu + 1)],
                                ctxT[:, wsl], start=True, stop=True)
                            if half == 1:
                                stg = stgp.tile([128, 2 * TQ], BF16,
                                                tag="stg", name=f"stg{mu}")
                                nc.vector.tensor_copy(stg[:], wp[:])
                                nc.gpsimd.dma_start(
                                    out=outT_r[:, 2 * pair:2 * pair + 2, wsl],
                                    in_=stg.rearrange("p (m t) -> p m t", m=2))

                        for pair in range(4):
                            for half in range(2):
                                blocks.append(
                                    lambda p=pair, h=half: wo_mm(p, h))

                    # --- attention groups for chunk a ---
                    if a is not None:
                        ot = [ps_ot.tile([65, TQ], F32, tag="ot",
                                         name=f"ot{h}_{a}")
                              for h in range(HPC)]
                        first_pv = [True, True]

                        def pv(h, r, esrc, osl, stop):
                            nc.tensor.matmul(
                                ot[h][:, osl], V_sb[:, h, r, 0:65], esrc,
                                start=first_pv[h], stop=stop)
                            first_pv[h] = False

                        def offdiag(h, p_idx, asl=asl):
                            r0, r1 = 2 * p_idx, 2 * p_idx + 1
                            hsl = slice(64 * h, 64 * (h + 1))
                            st = ps_st.tile([128, 2 * TQ], F32, tag="st",
                                            name=f"st{h}_{p_idx}")
                            for rl, r in ((0, r0), (1, r1)):
                                nc.tensor.matmul(
                                    st[:, TQ * rl:TQ * (rl + 1)],
                                    KTn[hsl, 128 * r:128 * (r + 1)],
                                    QTn[hsl, asl], start=True, stop=True)
                            e = ep.tile([128, 2 * TQ], BF16, tag="e1",
                                        name=f"e{h}_{p_idx}")
                            nc.scalar.activation(e[:], st[:], AF.Exp,
                                                 scale=SCALE)
                            for rl, r in ((0, r0), (1, r1)):
                                pv(h, r, e[:, TQ * rl:TQ * (rl + 1)],
                                   slice(0, TQ), False)

                        for p_idx in range(2 * a):
                            for h in range(HPC):
                                groups.append(
                                    lambda h=h, p=p_idx: offdiag(h, p))

                        def band(h, a=a, asl=asl):
                            # ragged diagonal: r-tile 4a+s covers tq>=128s
                            hsl = slice(64 * h, 64 * (h + 1))
                            r = 4 * a

                            def kt(s):
                                return KTn[hsl,
                                           128 * (r + s):128 * (r + s + 1)]

                            def qt(s):
                                return QTn[hsl, TQ * a + 128 * s:TQ * (a + 1)]

                            stb = ps_st.tile([128, 2 * TQ], F32, tag="st",
                                             name=f"stb{h}_{a}")
                            sb2 = ps_acc.tile([128, TQ], F32, tag="acc",
                                              name=f"sb2{h}_{a}")
                            nc.tensor.matmul(stb[:, 0:512], kt(0), qt(0),
                                             start=True, stop=True)
                            nc.tensor.matmul(stb[:, 512:896], kt(1), qt(1),
                                             start=True, stop=True)
                            nc.tensor.matmul(stb[:, 896:1024], kt(3), qt(3),
                                             start=True, stop=True)
                            nc.tensor.matmul(sb2[:, 0:256], kt(2), qt(2),
                                             start=True, stop=True)
                            e1 = ep.tile([128, 2 * TQ], BF16, tag="e1",
                                         name=f"eb{h}_{a}")
                            nc.scalar.activation(e1[:], stb[:], AF.Exp,
                                                 scale=SCALE)
                            e2 = ep.tile([128, 256], BF16, tag="e2",
                                         name=f"eb2{h}_{a}")
                            nc.scalar.activation(e2[:], sb2[:, 0:256], AF.Exp,
                                                 scale=SCALE)
                            # triangle masks on the four true-diagonal blocks
                            for blk in (e1[:, 0:128], e1[:, 512:640],
                                        e1[:, 896:1024], e2[:, 0:128]):
                                nc.vector.tensor_mul(blk, blk, tri_sb[:])
                            pv(h, r, e1[:, 0:512], slice(0, TQ), False)
                            pv(h, r + 1, e1[:, 512:896], slice(128, TQ),
                               False)
                            pv(h, r + 2, e2[:, 0:256], slice(256, TQ), False)
                            pv(h, r + 3, e1[:, 896:1024], slice(384, TQ),
                               True)

                        for h in range(HPC):
                            groups.append(lambda h=h: band(h))

                    # --- emit: b2 first, then interleave blocks & groups ---
                    if w is not None:
                        b2 = ps_acc.tile([128, TQ], F32, tag="acc",
                                         name=f"b2_{w}")
                        nc.tensor.matmul(b2[:], sel2_sb[:], recs[:, wsl],
                                         start=True, stop=True)
                        nc.vector.tensor_mul(ctxT[:, wsl], ctxU[:, wsl], b2[:])

                    bi = gi = 0
                    while bi < len(blocks) or gi < len(groups):
                        if bi < len(blocks):
                            blocks[bi]()
                            bi += 1
                        if gi < len(groups):
                            groups[gi]()
                            gi += 1

                    # --- softmax denominators + unnormalized ctx staging ---
                    if a is not None:
                        for h in range(HPC):
                            lnd = lnp.tile([1, TQ], F32, tag="lnd",
                                           name=f"lnd{h}")
                            nc.scalar.activation(lnd[:], ot[h][64:65, :],
                                                 AF.Ln, bias=zero_sb[:])
                            nc.scalar.activation(
                                recs[32 * h:32 * h + 1, asl], lnd[:],
                                AF.Exp, scale=-1.0)
                        for h in range(HPC):
                            nc.vector.tensor_copy(
                                ctxU[64 * h:64 * (h + 1), asl],
                                ot[h][0:64, :])

                    # --- broadcast 1/rms over partitions; normalize Q,K ---
                    if c is not None:
                        bb_q = ps_acc.tile([128, TQ], F32, tag="acc",
                                           name=f"bbq{c}")
                        nc.tensor.matmul(bb_q[:], selq_sb[:], rec_q[:, csl],
                                         start=True, stop=True)
                        nc.vector.tensor_mul(QTn[:, csl], state["raw_q"][:],
                                             bb_q[:])
                        bb_k = ps_acc.tile([128, TQ], F32, tag="acc",
                                           name=f"bbk{c}")
                        nc.tensor.matmul(bb_k[:], selk_sb[:], rec_k[:, csl],
                                         start=True, stop=True)
                        nc.vector.tensor_mul(KTn[:, csl], state["raw_k"][:],
                                             bb_k[:])

    nc.compile()
    return nc


_NC_CACHE = None


def _get_nc():
    global _NC_CACHE
    if _NC_CACHE is None:
        _NC_CACHE = build_nc()
    return _NC_CACHE


def _make_in_maps(x, w_q, w_k, w_v, w_o, q_gamma, k_gamma):
    x = np.asarray(x, dtype=np.float32)
    xT = np.ascontiguousarray(x.reshape(T, C).T).astype(NP_BF16)  # [C, T]

    p = np.arange(128)
    gq = np.tile(np.asarray(q_gamma, np.float32), HPC)   # [128]
    gk = np.tile(np.asarray(k_gamma, np.float32), HPC)
    selq = np.zeros((2, 128), np.float32)
    selk = np.zeros((2, 128), np.float32)
    sel2 = np.zeros((64, 128), np.float32)
    for h in range(HPC):
        blk = (p // 64 == h)
        selq[h] = blk * gq
        selk[h] = blk * gk
        sel2[32 * h] = blk
    onescol = np.ascontiguousarray(
        np.stack([(p // 64 == h).astype(np.float32)
                  for h in range(HPC)]).T)
    f = np.arange(128)
    tri = (f[None, :] >= p[:, None]).astype(NP_BF16)

    common = dict(xT=xT,
                  selq=selq.astype(NP_BF16), selk=selk.astype(NP_BF16),
                  sel2=sel2.astype(NP_BF16),
                  onescol=onescol.astype(NP_BF16), tri=tri,
                  vones=np.ones((128, 32), dtype=NP_BF16))

    in_maps = []
    for i in range(NCORES):
        rows = slice(JPC * i, JPC * (i + 1))
        wqkv = np.concatenate(
            [np.asarray(w_q, np.float32)[rows].T,
             np.asarray(w_k, np.float32)[rows].T,
             np.asarray(w_v, np.float32)[rows].T], axis=1)  # [C, 384]
        wo = np.asarray(w_o, np.float32)[:, rows].T          # [128, C]
        in_maps.append(dict(common,
                            wqkv=np.ascontiguousarray(wqkv).astype(NP_BF16),
                            wo=np.ascontiguousarray(wo).astype(NP_BF16)))
    return in_maps


def _run(x, w_q, w_k, w_v, w_o, q_gamma, k_gamma, trace=False):
    import time

    from concourse.bass_utils import run_bass_kernel_spmd
    nc = _get_nc()
    in_maps = _make_in_maps(x, w_q, w_k, w_v, w_o, q_gamma, k_gamma)
    res = None
    for attempt in range(3):
        try:
            res = run_bass_kernel_spmd(nc, in_maps, list(range(NCORES)),
                                       trace=trace)
            break
        except Exception:
            # rare transient NRT_EXEC_UNIT_UNRECOVERABLE under axon; the
            # terminal resets the device on the next load
            if attempt == 2:
                raise
            time.sleep(3.0)
    acc = np.zeros((C, T), dtype=np.float32)
    for r in res.results:
        acc += r["outT"].astype(np.float32)
    out = acc.T.astype(np.float32).reshape(1, T, C)
    return out, res


def kernel(x, w_q, w_k, w_v, w_o, q_gamma, k_gamma):
    out, _ = _run(x, w_q, w_k, w_v, w_o, q_gamma, k_gamma, trace=False)
    return out


# revision 32
# speedup vs baseline: 1.1383x; 1.1383x over previous
"""Trainium2 Bass kernel for nn_Attention_64819646431478.

Single-layer causal attention, B=1, T=2048, DIM=1024, 16 heads, head_dim=64,
f32, with RMSNorm (eps=f32 eps) on Q and K heads.

Sharding: tensor-parallel over heads across 8 NeuronCores (2 heads/core).
Each core computes its heads' Q/K/V projections, causal attention, and the
partial output projection against its 128-row slice of w_o; the host sums
the 8 bf16 partial outputs (the "all-reduce" of the hint, at gather time).

v2 layout (fused pipeline; ~2x over the phase-split baseline):
  - One fused loop, software-pipelined: iteration i runs QKV projections of
    chunk c=i, attention of chunk a=i-1, and w_o of chunk w=i-2, so the PE
    never idles at a phase boundary (idle >3.4us re-throttles HAM to 1.2GHz).
  - Single ACT table set (natural_log_exp_and_others): rsqrt for RMSNorm is
    exp(-0.5*ln(mean+eps)); softmax reciprocal is exp(-ln(sum)); plus Square,
    Exp, Copy. No DVE reciprocal passes, no table reloads mid-kernel.
  - Causal band is ragged: diagonal r-tile s covers only tq>=128*s (widths
    512/384/256/128), packed into 1.25 PSUM score tiles; only the four
    128x128 blocks on the true diagonal get a triangle mask (DVE, bf16).
  - Scores are computed transposed per head: ST[tk,tq] = K^ @ Q^, exp needs
    no max-subtraction (|scores|<=8 after RMSNorm).
  - Softmax denominator rides along as V's 65th lhsT column (ones), landing
    in ot row 64 of the PV accumulation.
  - V is transposed by the DMA crossbar (dma_start_transpose), not the PE.
  - PSUM budget exactly 8 banks: st [128,1024]x2 (scores + w_o pairs),
    acc [*,512]x3 (sums/ot0/ot1/b2/bb_q/bb_k rotation), pp [128,512]x1.
  - Output partials are written bf16 (host accumulates in f32).
"""

import os
import sys
import types

import numpy as np

# --- environment bootstrap (harness may run us from a bare directory) ---
for _p in ("/root/.axon_site", "/root/.axon_site/_ro/trn_rl_repo",
           "/root/.axon_site/_ro/pypackages", "/opt/trn_rl_repo"):
    if os.path.isdir(_p) and _p not in sys.path:
        sys.path.append(_p)


def _install_ntff_shim():
    """Provide antenv.axon_hooks (missing in this image) so trace=True works."""
    if "antenv.axon_hooks" in sys.modules:
        return
    mod = types.ModuleType("antenv.axon_hooks")
    mod._hook = None
    mod.set_axon_ntff_profile_hook = lambda h: setattr(mod, "_hook", h)
    mod.get_axon_ntff_profile_hook = lambda: mod._hook
    sys.modules["antenv.axon_hooks"] = mod
    try:
        import antenv
        antenv.axon_hooks = mod
        from trn_agent_boot.trn_boot import _ntff_profile_via_ctypes
        mod.set_axon_ntff_profile_hook(
            _ntff_profile_via_ctypes("/opt/axon/libaxon_pjrt.so"))
    except Exception:
        pass


_install_ntff_shim()

import ml_dtypes  # noqa: E402

import concourse.mybir as mybir  # noqa: E402
import concourse.tile as tile  # noqa: E402
from concourse import bacc  # noqa: E402

F32 = mybir.dt.float32
BF16 = mybir.dt.bfloat16
NP_BF16 = ml_dtypes.bfloat16
AF = mybir.ActivationFunctionType

_TABLES_PATCHED = False


def _pin_act_table_set():
    """Make the ACT table-load chooser resolve Exp/Ln/Square/Copy to the one
    set that holds all four (natural_log_exp_and_others). The default
    per-function primary sets differ (exp_and_others vs natural_log), so a
    kernel alternating Exp and Ln reloads tables every few instructions
    (~2.7us each, 33 loads measured). Stripping these funcs from every other
    set — order and indices preserved — leaves the chooser exactly one valid
    set, so it emits a single load."""
    global _TABLES_PATCHED
    if _TABLES_PATCHED:
        return
    import functools

    import concourse.bacc as bacc_mod
    from concourse.hw_specs import get_activation_tables as _orig

    keep = {AF.Exp, AF.Ln, AF.Square, AF.Copy}
    target = "natural_log_exp_and_others"

    @functools.lru_cache(maxsize=None)
    def patched(arch):
        tabs = _orig(arch)
        if target not in tabs or not keep.issubset(tabs[target]):
            return tabs
        return {name: (funcs if name == target else funcs - keep)
                for name, funcs in tabs.items()}

    bacc_mod.get_activation_tables = patched
    _TABLES_PATCHED = True

T = 2048
C = 1024
D = 64
NCORES = 8
HPC = 2            # heads per core
JPC = HPC * D      # 128 j-columns per core
NTQ = 4            # tq chunks of 512
TQ = 512
EPS = float(np.finfo(np.float32).eps)
SCALE = float(D) ** -0.5


def build_nc():
    _pin_act_table_set()
    nc = bacc.Bacc("TRN2", target_bir_lowering=False, debug=False,
                   num_devices=NCORES)

    xT_d = nc.dram_tensor("xT", [C, T], BF16, kind="ExternalInput")
    wqkv_d = nc.dram_tensor("wqkv", [C, 3 * JPC], BF16, kind="ExternalInput")
    wo_d = nc.dram_tensor("wo", [JPC, C], BF16, kind="ExternalInput")
    selq_d = nc.dram_tensor("selq", [2, 128], BF16, kind="ExternalInput")
    selk_d = nc.dram_tensor("selk", [2, 128], BF16, kind="ExternalInput")
    sel2_d = nc.dram_tensor("sel2", [64, 128], BF16, kind="ExternalInput")
    onescol_d = nc.dram_tensor("onescol", [128, 2], BF16, kind="ExternalInput")
    tri_d = nc.dram_tensor("tri", [128, 128], BF16, kind="ExternalInput")
    vones_d = nc.dram_tensor("vones", [128, 32], BF16, kind="ExternalInput")
    outT_d = nc.dram_tensor("outT", [C, T], BF16, kind="ExternalOutput")

    with tile.TileContext(nc) as tc, nc.allow_low_precision("bf16 kernel"):
        from contextlib import ExitStack
        with ExitStack() as ctx:
            consts = ctx.enter_context(tc.tile_pool(name="consts", bufs=1))
            acts = ctx.enter_context(tc.tile_pool(name="acts", bufs=1))

            # ---- constants to SBUF ----
            wsb = consts.tile([128, 8, 3 * JPC], BF16)
            nc.gpsimd.dma_start(
                out=wsb[:], in_=wqkv_d.rearrange("(c p) j -> p c j", p=128))
            wo_sb = consts.tile([128, C], BF16)
            nc.sync.dma_start(out=wo_sb[:], in_=wo_d[:])
            selq_sb = consts.tile([2, 128], BF16)
            nc.sync.dma_start(out=selq_sb[:], in_=selq_d[:])
            selk_sb = consts.tile([2, 128], BF16)
            nc.sync.dma_start(out=selk_sb[:], in_=selk_d[:])
            sel2_sb = consts.tile([64, 128], BF16)
            nc.sync.dma_start(out=sel2_sb[:], in_=sel2_d[:])
            onescol_sb = consts.tile([128, 2], BF16)
            nc.sync.dma_start(out=onescol_sb[:], in_=onescol_d[:])
            tri_sb = consts.tile([128, 128], BF16)
            nc.sync.dma_start(out=tri_sb[:], in_=tri_d[:])
            eps_sb = consts.tile([2, 1], F32)
            nc.vector.memset(eps_sb[:], EPS)
            zero_sb = consts.tile([1, 1], F32)
            nc.vector.memset(zero_sb[:], 0.0)

            # ---- persistent activations ----
            xT_sb = acts.tile([128, 8, T], BF16)
            QTn = acts.tile([128, T], BF16)
            KTn = acts.tile([128, T], BF16)
            V_sb = acts.tile([128, HPC, 16, 65], BF16)  # [tk%128, h, r, d|1]
            ctxU = acts.tile([128, T], BF16)   # unnormalized ctx^T
            ctxT = acts.tile([128, T], BF16)   # normalized ctx^T
            rec_q = acts.tile([2, T], BF16)    # 1/rms per q head
            rec_k = acts.tile([2, T], BF16)    # 1/rms per k head
            # 1/softmax-denominator: h0 at row 0, h1 at row 32 (engine
            # partition starts must be 32-aligned); other rows zeroed.
            recs = acts.tile([64, T], BF16)
            nc.vector.memset(recs[:], 0.0)

            # ones column of each V tile (lhsT col 64 -> softmax sums row)
            nc.sync.dma_start(
                out=V_sb[:, :, :, 64:65],
                in_=vones_d.rearrange("p (h r u) -> p h r u", h=HPC, u=1))

            # input x, chunk-major so chunk 0 projections start early;
            # split across two DMA queues to halve the ramp
            xT_r = xT_d.rearrange("(c p) t -> p c t", p=128)
            for c in range(NTQ):
                sl = slice(TQ * c, TQ * (c + 1))
                for ci in range(8):
                    eng = nc.gpsimd if ci % 2 == 0 else nc.sync
                    eng.dma_start(out=xT_sb[:, ci, sl], in_=xT_r[:, ci, sl])

            outT_r = outT_d.rearrange("(m p) t -> p m t", p=128)

            with (
                tc.tile_pool(name="vtp", bufs=2) as vtp,
                tc.tile_pool(name="sqp", bufs=2) as sqp,
                tc.tile_pool(name="rawp", bufs=2) as rawp,
                tc.tile_pool(name="lnp", bufs=2) as lnp,
                tc.tile_pool(name="ep", bufs=3) as ep,
                tc.tile_pool(name="stgp", bufs=2) as stgp,
                tc.tile_pool(name="ps_st", bufs=2, space="PSUM") as ps_st,
                tc.tile_pool(name="ps_acc", bufs=2, space="PSUM") as ps_acc,
                tc.tile_pool(name="ps_ot", bufs=2, space="PSUM") as ps_ot,
            ):
                def proj(jbase, csl, name):
                    pp = ps_acc.tile([128, TQ], F32, tag="acc", name=name)
                    for ci in range(8):
                        nc.tensor.matmul(
                            pp[:], wsb[:, ci, jbase:jbase + 128],
                            xT_sb[:, ci, csl], start=(ci == 0), stop=(ci == 7))
                    return pp

                pending_bb = {}
                for i in range(NTQ + 2):
                    c = i if i < NTQ else None
                    a = i - 1 if 1 <= i <= NTQ else None
                    w = i - 2 if i >= 2 else None
                    csl = slice(TQ * c, TQ * (c + 1)) if c is not None else None
                    asl = slice(TQ * a, TQ * (a + 1)) if a is not None else None
                    wsl = slice(TQ * w, TQ * (w + 1)) if w is not None else None

                    # Emission: chunk i-1's rms-broadcast + Q/K normalize
                    # first (its ACT deps finished last iteration), then
                    # projection blocks back-to-back (keeps the PE stream
                    # chunky so HAM stays warm), then ACT-paced attention
                    # groups with w_o matmuls as fillers.
                    blocks = []   # projection/sums PE blocks (run upfront)
                    wo_fill = []  # w_o fillers (one per attention group)
                    groups = []   # attention groups (score+exp+PV)
                    state = {}

                    if pending_bb:
                        pc, pcsl, praw_q, praw_k = pending_bb.pop("v")
                        bb_q = ps_acc.tile([128, TQ], F32, tag="acc",
                                           name=f"bbq{pc}")
                        nc.tensor.matmul(bb_q[:], selq_sb[:], rec_q[:, pcsl],
                                         start=True, stop=True)
                        nc.vector.tensor_mul(QTn[:, pcsl], praw_q[:], bb_q[:])
                        bb_k = ps_acc.tile([128, TQ], F32, tag="acc",
                                           name=f"bbk{pc}")
                        nc.tensor.matmul(bb_k[:], selk_sb[:], rec_k[:, pcsl],
                                         start=True, stop=True)
                        nc.vector.tensor_mul(KTn[:, pcsl], praw_k[:], bb_k[:])

                    if c is not None:
                        def proj_q(c=c, csl=csl):
                            state["pp_q"] = proj(0, csl, f"ppq{c}")
                            raw_q = rawp.tile([128, TQ], BF16, tag="raw",
                                              name="rawq")
                            nc.vector.tensor_copy(raw_q[:], state["pp_q"][:])
                            sq_q = sqp.tile([128, TQ], BF16, tag="sq",
                                            name="sqq")
                            nc.vector.tensor_mul(sq_q[:], raw_q[:], raw_q[:])
                            state["raw_q"], state["sq_q"] = raw_q, sq_q

                        def proj_k(c=c, csl=csl):
                            state["pp_k"] = proj(JPC, csl, f"ppk{c}")
                            raw_k = rawp.tile([128, TQ], BF16, tag="raw",
                                              name="rawk")
                            nc.vector.tensor_copy(raw_k[:], state["pp_k"][:])
                            sq_k = sqp.tile([128, TQ], BF16, tag="sq",
                                            name="sqk")
                            nc.vector.tensor_mul(sq_k[:], raw_k[:], raw_k[:])
                            state["raw_k"], state["sq_k"] = raw_k, sq_k

                        def proj_v(c=c, csl=csl):
                            pp_v = proj(2 * JPC, csl, f"ppv{c}")
                            vt = vtp.tile([128, TQ], BF16, tag="vt",
                                          name=f"vt{c}")
                            nc.vector.tensor_copy(vt[:], pp_v[:])
                            for rl in range(4):
                                # xbar transpose needs a plain contiguous
                                # dest; DVE splits the heads into V_sb
                                vx = vtp.tile([128, 128], BF16, tag="vx",
                                              name=f"vx{c}_{rl}")
                                nc.sync.dma_start_transpose(
                                    out=vx[:],
                                    in_=vt[:, 128 * rl:128 * (rl + 1)])
                                for h in range(HPC):
                                    nc.vector.tensor_copy(
                                        V_sb[:, h, 4 * c + rl, 0:64],
                                        vx[:, 64 * h:64 * (h + 1)])

                        def sums_blk(c=c, csl=csl):
                            # rms: rec = (mean+eps)^-1/2 = exp(-ln/2)
                            sums_q = ps_acc.tile([2, TQ], F32, tag="acc",
                                                 name=f"sumsq{c}")
                            nc.tensor.matmul(sums_q[:], onescol_sb[:],
                                             state["sq_q"][:],
                                             start=True, stop=True)
                            sums_k = ps_acc.tile([2, TQ], F32, tag="acc",
                                                 name=f"sumsk{c}")
                            nc.tensor.matmul(sums_k[:], onescol_sb[:],
                                             state["sq_k"][:],
                                             start=True, stop=True)
                            ln_q = lnp.tile([2, TQ], F32, tag="lns",
                                            name="lnq")
                            nc.scalar.activation(ln_q[:], sums_q[:], AF.Ln,
                                                 bias=eps_sb[:], scale=1.0 / D)
                            nc.scalar.activation(rec_q[:, csl], ln_q[:],
                                                 AF.Exp, scale=-0.5)
                            ln_k = lnp.tile([2, TQ], F32, tag="lns",
                                            name="lnk")
                            nc.scalar.activation(ln_k[:], sums_k[:], AF.Ln,
                                                 bias=eps_sb[:], scale=1.0 / D)
                            nc.scalar.activation(rec_k[:, csl], ln_k[:],
                                                 AF.Exp, scale=-0.5)

                        blocks += [proj_q, proj_k, proj_v, sums_blk]

                    if w is not None:
                        wstate = {}

                        def wo_mm(pair, half, wsl=wsl, w=w, wstate=wstate):
                            mu = 2 * pair + half
                            if half == 0:
                                wstate[pair] = ps_st.tile(
                                    [128, 2 * TQ], F32, tag="st",
                                    name=f"wp{w}_{pair}")
                            wp = wstate[pair]
                            nc.tensor.matmul(
                                wp[:, TQ * half:TQ * (half + 1)],
                                wo_sb[:, 128 * mu:128 * (mu + 1)],
                                ctxT[:, wsl], start=True, stop=True)
                            if half == 1:
                                stg = stgp.tile([128, 2 * TQ], BF16,
                                                tag="stg", name=f"stg{mu}")
                                nc.vector.tensor_copy(stg[:], wp[:])
                                nc.gpsimd.dma_start(
                                    out=outT_r[:, 2 * pair:2 * pair + 2, wsl],
                                    in_=stg.rearrange("p (m t) -> p m t", m=2))

                        for pair in range(4):
                            for half in range(2):
                                wo_fill.append(
                                    lambda p=pair, h=half: wo_mm(p, h))

                    # --- attention groups for chunk a ---
                    if a is not None:
                        ot = [ps_ot.tile([65, TQ], F32, tag="ot",
                                         name=f"ot{h}_{a}")
                              for h in range(HPC)]
                        first_pv = [True, True]

                        def pv(h, r, esrc, osl, stop):
                            nc.tensor.matmul(
                                ot[h][:, osl], V_sb[:, h, r, 0:65], esrc,
                                start=first_pv[h], stop=stop)
                            first_pv[h] = False

                        def offdiag(h, p_idx, asl=asl):
                            r0, r1 = 2 * p_idx, 2 * p_idx + 1
                            hsl = slice(64 * h, 64 * (h + 1))
                            st = ps_st.tile([128, 2 * TQ], F32, tag="st",
                                            name=f"st{h}_{p_idx}")
                            for rl, r in ((0, r0), (1, r1)):
                                nc.tensor.matmul(
                                    st[:, TQ * rl:TQ * (rl + 1)],
                                    KTn[hsl, 128 * r:128 * (r + 1)],
                                    QTn[hsl, asl], start=True, stop=True)
                            e = ep.tile([128, 2 * TQ], BF16, tag="e1",
                                        name=f"e{h}_{p_idx}")
                            nc.scalar.activation(e[:], st[:], AF.Exp,
                                                 scale=SCALE)
                            for rl, r in ((0, r0), (1, r1)):
                                pv(h, r, e[:, TQ * rl:TQ * (rl + 1)],
                                   slice(0, TQ), False)

                        for p_idx in range(2 * a):
                            for h in range(HPC):
                                groups.append(
                                    lambda h=h, p=p_idx: offdiag(h, p))

                        def band(h, a=a, asl=asl):
                            # ragged diagonal: r-tile 4a+s covers tq>=128s
                            hsl = slice(64 * h, 64 * (h + 1))
                            r = 4 * a

                            def kt(s):
                                return KTn[hsl,
                                           128 * (r + s):128 * (r + s + 1)]

                            def qt(s):
                                return QTn[hsl, TQ * a + 128 * s:TQ * (a + 1)]

                            stb = ps_st.tile([128, 2 * TQ], F32, tag="st",
                                             name=f"stb{h}_{a}")
                            sb2 = ps_acc.tile([128, TQ], F32, tag="acc",
                                              name=f"sb2{h}_{a}")
                            nc.tensor.matmul(stb[:, 0:512], kt(0), qt(0),
                                             start=True, stop=True)
                            nc.tensor.matmul(stb[:, 512:896], kt(1), qt(1),
                                             start=True, stop=True)
                            nc.tensor.matmul(stb[:, 896:1024], kt(3), qt(3),
                                             start=True, stop=True)
                            nc.tensor.matmul(sb2[:, 0:256], kt(2), qt(2),
                                             start=True, stop=True)
                            e1 = ep.tile([128, 2 * TQ], BF16, tag="e1",
                                         name=f"eb{h}_{a}")
                            nc.scalar.activation(e1[:], stb[:], AF.Exp,
                                                 scale=SCALE)
                            e2 = ep.tile([128, 256], BF16, tag="e2",
                                         name=f"eb2{h}_{a}")
                            nc.scalar.activation(e2[:], sb2[:, 0:256], AF.Exp,
                                                 scale=SCALE)
                            # triangle masks on the four true-diagonal blocks
                            for blk in (e1[:, 0:128], e1[:, 512:640],
                                        e1[:, 896:1024], e2[:, 0:128]):
                                nc.vector.tensor_mul(blk, blk, tri_sb[:])
                            pv(h, r, e1[:, 0:512], slice(0, TQ), False)
                            pv(h, r + 1, e1[:, 512:896], slice(128, TQ),
                               False)
                            pv(h, r + 2, e2[:, 0:256], slice(256, TQ), False)
                            pv(h, r + 3, e1[:, 896:1024], slice(384, TQ),
                               True)

                        for h in range(HPC):
                            groups.append(lambda h=h: band(h))

                    # --- emit: b2, then proj blocks, then groups+fillers ---
                    if w is not None:
                        b2 = ps_acc.tile([128, TQ], F32, tag="acc",
                                         name=f"b2_{w}")
                        nc.tensor.matmul(b2[:], sel2_sb[:], recs[:, wsl],
                                         start=True, stop=True)
                        nc.vector.tensor_mul(ctxT[:, wsl], ctxU[:, wsl], b2[:])

                    for blk_fn in blocks:
                        blk_fn()
                    for g_fn in groups:
                        g_fn()
                        if wo_fill:
                            wo_fill.pop(0)()
                    while wo_fill:
                        wo_fill.pop(0)()

                    # --- softmax denominators + unnormalized ctx staging ---
                    if a is not None:
                        for h in range(HPC):
                            lnd = lnp.tile([1, TQ], F32, tag="lnd",
                                           name=f"lnd{h}")
                            nc.scalar.activation(lnd[:], ot[h][64:65, :],
                                                 AF.Ln, bias=zero_sb[:])
                            nc.scalar.activation(
                                recs[32 * h:32 * h + 1, asl], lnd[:],
                                AF.Exp, scale=-1.0)
                        for h in range(HPC):
                            nc.vector.tensor_copy(
                                ctxU[64 * h:64 * (h + 1), asl],
                                ot[h][0:64, :])

                    # --- defer 1/rms broadcast + Q/K normalize to the start
                    # of the next iteration (exp_rms is surely drained) ---
                    if c is not None:
                        pending_bb["v"] = (c, csl, state["raw_q"],
                                           state["raw_k"])

    nc.compile()
    return nc


_NC_CACHE = None


def _get_nc():
    global _NC_CACHE
    if _NC_CACHE is None:
        _NC_CACHE = build_nc()
    return _NC_CACHE


def _make_in_maps(x, w_q, w_k, w_v, w_o, q_gamma, k_gamma):
    x = np.asarray(x, dtype=np.float32)
    xT = np.ascontiguousarray(x.reshape(T, C).T).astype(NP_BF16)  # [C, T]

    p = np.arange(128)
    gq = np.tile(np.asarray(q_gamma, np.float32), HPC)   # [128]
    gk = np.tile(np.asarray(k_gamma, np.float32), HPC)
    selq = np.zeros((2, 128), np.float32)
    selk = np.zeros((2, 128), np.float32)
    sel2 = np.zeros((64, 128), np.float32)
    for h in range(HPC):
        blk = (p // 64 == h)
        selq[h] = blk * gq
        selk[h] = blk * gk
        sel2[32 * h] = blk
    onescol = np.ascontiguousarray(
        np.stack([(p // 64 == h).astype(np.float32)
                  for h in range(HPC)]).T)
    f = np.arange(128)
    tri = (f[None, :] >= p[:, None]).astype(NP_BF16)

    common = dict(xT=xT,
                  selq=selq.astype(NP_BF16), selk=selk.astype(NP_BF16),
                  sel2=sel2.astype(NP_BF16),
                  onescol=onescol.astype(NP_BF16), tri=tri,
                  vones=np.ones((128, 32), dtype=NP_BF16))

    in_maps = []
    for i in range(NCORES):
        rows = slice(JPC * i, JPC * (i + 1))
        wqkv = np.concatenate(
            [np.asarray(w_q, np.float32)[rows].T,
             np.asarray(w_k, np.float32)[rows].T,
             np.asarray(w_v, np.float32)[rows].T], axis=1)  # [C, 384]
        wo = np.asarray(w_o, np.float32)[:, rows].T          # [128, C]
        in_maps.append(dict(common,
                            wqkv=np.ascontiguousarray(wqkv).astype(NP_BF16),
                            wo=np.ascontiguousarray(wo).astype(NP_BF16)))
    return in_maps


def _run(x, w_q, w_k, w_v, w_o, q_gamma, k_gamma, trace=False):
    import time

    from concourse.bass_utils import run_bass_kernel_spmd
    nc = _get_nc()
    in_maps = _make_in_maps(x, w_q, w_k, w_v, w_o, q_gamma, k_gamma)
    res = None
    for attempt in range(3):
        try:
            res = run_bass_kernel_spmd(nc, in_maps, list(range(NCORES)),
                                       trace=trace)
            break
        except Exception:
            # rare transient NRT_EXEC_UNIT_UNRECOVERABLE under axon; the
            # terminal resets the device on the next load
            if attempt == 2:
                raise
            time.sleep(3.0)
    acc = np.zeros((C, T), dtype=np.float32)
    for r in res.results:
        acc += r["outT"].astype(np.float32)
    out = acc.T.astype(np.float32).reshape(1, T, C)
    return out, res


def kernel(x, w_q, w_k, w_v, w_o, q_gamma, k_gamma):
    out, _ = _run(x, w_q, w_k, w_v, w_o, q_gamma, k_gamma, trace=False)
    return out
